# revision 13
# baseline (speedup 1.0000x reference)
"""CodeWiseAttention kernel for Trainium2 (8 NeuronCores, label-dim sharded).

m[b,n,:] = softmax(label_feature[n] @ x[b].T) @ x[b]

Sharding: label rows N=8922 split across 8 cores (1116/core, padded to 1152);
x replicated.  Per core, per batch b:
  mm1 (fp16):  S^T[l,n] = xT[e,l].T @ labT[e,n]     (xT, labT pre-transposed
               on host; fp16 runs the PE at 1 cycle/row vs ~3.5 for fp32)
  exp on ScalarE: expS = exp(S - 30) -> bf16        (constant shift; cancels)
  mm2 (bf16):  Uaug^T[e',n] += xa[l,e'].T @ expS^T[l,n]  accumulated over l,
               where xa has a ones column so row 100 of Uaug = Z = sum_l expS.
  out: DMA Uaug^T [101, n] to DRAM; host divides by Z and transposes.

PSUM layout (8 banks x 512 fp32 per partition), chosen so every matmul
output region sits inside one bank, accumulator banks are never touched by
score writes (start=True clears has_written for the WHOLE bank), and no
bank is PE-written while ScalarE/VectorE reads it (fatal collision; the
Tile tracker would serialize):
  b0 [0:512]      scores buf A, n[0:512]     } one contiguous 1024-wide
  b1 [512:1024]   scores buf A, n[512:1024]  }   ACTIVATE per l-chunk
  b2 [1024:1536]  scores buf B, n[0:512]
  b3 [1536:2048]  scores buf B, n[512:1024]
  b4 [2048:2560]  U accum, n[0:512]
  b5 [2560:3072]  U accum, n[512:1024]
  b6 [3072:3532]  "pack" scores: n[1024:1116] for 5 l-chunks at once
  b7 [3584:3676]  U accum, n[1024:1116]
The 92-wide n[1024:1116] tail is handled in 4 packs of 5 l-chunks so its
exp also runs as a few large ACTIVATEs (~293ns fixed cost per ACTIVATE).
"""
import numpy as np
import ml_dtypes
from contextlib import ExitStack

import concourse.tile as tile
from concourse import bacc, mybir
from concourse.bass_utils import run_bass_kernel_spmd

F32 = mybir.dt.float32
F16 = mybir.dt.float16
BF16 = mybir.dt.bfloat16

B, L, E = 8, 2500, 100
LP = 2520          # L padded; pad rows have xa=0 (incl. ones col) so they
                   # contribute nothing to U or Z even though exp(0-30) != 0
N_TOTAL = 8922
NCORES = 8
NS = 1116          # label rows per core (core 7: 1110 real)
NSP = 1116         # per-core label columns (no padding: 1024 main + 92 tail)
NMAIN = 1024       # n columns handled by the main (512,512) loop
NT = 92            # tail n columns handled by packs
LC = 126           # l-chunk rows (partition dim of S^T)
NLC = LP // LC     # 20 l-chunks
PACK = 5           # l-chunks per tail pack (5*92*4B fits one PSUM bank)
NPACK = NLC // PACK
EA = E + 1         # x augmented with ones column
EXP_BIAS = -30.0

SA0, SB0 = 0, 1024     # main score buffer offsets (f32 elems)
U0 = 2048              # main U accumulator offset
PS0 = 3072             # pack score offset
UT0 = 3584             # tail U accumulator offset

TRACE = False
LAST_RESULT = None

_NC = []


def _build():
    nc = bacc.Bacc("TRN2", target_bir_lowering=False, debug=False)
    xt_d = nc.dram_tensor("xt", [B, E, LP], F16, kind="ExternalInput").ap()
    # xa pre-rearranged on host to [B, LC, NLC, EA] so the DMA is contiguous
    xa_d = nc.dram_tensor("xa", [B, LC, NLC, EA], BF16,
                          kind="ExternalInput").ap()
    lab_d = nc.dram_tensor("labT", [E, NSP], F16, kind="ExternalInput").ap()
    m_d = nc.dram_tensor("m", [B, EA, NSP], F32, kind="ExternalOutput").ap()

    with tile.TileContext(nc) as tc, ExitStack() as ctx:
        consts = ctx.enter_context(tc.tile_pool(name="consts", bufs=1))
        xt_pool = ctx.enter_context(tc.tile_pool(name="xtp", bufs=2))
        xa_pool = ctx.enter_context(tc.tile_pool(name="xap", bufs=2))
        e_pool = ctx.enter_context(tc.tile_pool(name="ep", bufs=3))
        et_pool = ctx.enter_context(tc.tile_pool(name="etp", bufs=2))
        u_pool = ctx.enter_context(tc.tile_pool(name="up", bufs=2))
        ps = ctx.enter_context(tc.tile_pool(name="ps", bufs=1, space="PSUM"))

        arena = ps.tile([128, 4096], F32)

        labT = consts.tile([E, NSP], F16)
        nc.sync.dma_start(out=labT[:], in_=lab_d)
        bias_sb = consts.tile([128, 1], F32)
        nc.vector.memset(bias_sb[:], EXP_BIAS)

        xt_tiles, xa_tiles = {}, {}

        def fetch(b):
            xt_tiles[b] = xt_pool.tile([E, LP], F16, tag="xt", name=f"xt{b}")
            nc.sync.dma_start(out=xt_tiles[b][:], in_=xt_d[b])
            xa_tiles[b] = xa_pool.tile(
                [LC, NLC, EA], BF16, tag="xa", name=f"xa{b}")
            nc.sync.dma_start(out=xa_tiles[b][:], in_=xa_d[b])

        fetch(0)
        prev_tail = [None]
        for b in range(B):
            xT = xt_tiles.pop(b)
            xa_sb = xa_tiles.pop(b)
            if b + 1 < B:
                fetch(b + 1)

            e_sbs, pe_sbs = {}, {}

            def mm1(xt_tile, c):
                base = SA0 if c % 2 == 0 else SB0
                for j in range(2):
                    mi = nc.tensor.matmul(
                        arena[:LC, base + j * 512:base + (j + 1) * 512],
                        xt_tile[:, c * LC:(c + 1) * LC],
                        labT[:, j * 512:(j + 1) * 512],
                    )
                    if j == 1:
                        # same stationary weights as j==0: skip the reload
                        mi.ins.ldweights = False

            def act(c):
                base = SA0 if c % 2 == 0 else SB0
                e_sb = e_pool.tile([128, NMAIN], BF16, tag="e", name=f"e{c}")
                nc.scalar.activation(
                    e_sb[:LC, :], arena[:LC, base:base + NMAIN],
                    mybir.ActivationFunctionType.Exp,
                    bias=bias_sb[:LC], scale=1.0,
                )
                e_sbs[c] = e_sb

            def mm2(c):
                e_sb = e_sbs.pop(c)
                for j in range(2):
                    mi = nc.tensor.matmul(
                        arena[:EA, U0 + j * 512:U0 + (j + 1) * 512],
                        xa_sb[:, c, :],
                        e_sb[:LC, j * 512:(j + 1) * 512],
                        start=(c == 0), stop=(c == NLC - 1),
                    )
                    if j == 1:
                        mi.ins.ldweights = False

            def pack_mm1(p):
                for i in range(PACK):
                    c = p * PACK + i
                    nc.tensor.matmul(
                        arena[:LC, PS0 + i * NT:PS0 + (i + 1) * NT],
                        xT[:, c * LC:(c + 1) * LC],
                        labT[:, NMAIN:NSP],
                    )

            def pack_act(p):
                pe = et_pool.tile([128, PACK * NT], BF16, tag="pe",
                                  name=f"pe{p}")
                nc.scalar.activation(
                    pe[:LC, :], arena[:LC, PS0:PS0 + PACK * NT],
                    mybir.ActivationFunctionType.Exp,
                    bias=bias_sb[:LC], scale=1.0,
                )
                pe_sbs[p] = pe

            def pack_mm2(p):
                pe = pe_sbs.pop(p)
                for i in range(PACK):
                    c = p * PACK + i
                    nc.tensor.matmul(
                        arena[:EA, UT0:UT0 + NT],
                        xa_sb[:, c, :],
                        pe[:LC, i * NT:(i + 1) * NT],
                        start=(c == 0), stop=(c == NLC - 1),
                    )

            # software-pipelined emission: the in-order PE never has a
            # ready mm1 queued behind an exp-waiting mm2, and pack mm2 is
            # deferred ~2 l-chunks so the pack exp is long done.  The LAST
            # pack's mm2 (plus the U-tail copy and the output DMA) is
            # deferred into the NEXT batch so the PE doesn't stall at the
            # batch boundary waiting for the final pack exp.
            pending = []
            if b == 0:
                mm1(xT, 0)
            for c in range(NLC):
                act(c)
                if c + 1 < NLC:
                    mm1(xT, c + 1)
                elif b + 1 < B:
                    # hoist next batch's first mm1 ahead of the boundary so
                    # its first exp starts with no gap in the ACT stream
                    mm1(xt_tiles[b + 1], 0)
                mm2(c)
                if c == 0 and prev_tail[0] is not None:
                    prev_tail[0]()
                    prev_tail[0] = None
                if c % PACK == PACK - 1:
                    p = c // PACK
                    pack_mm1(p)
                    pack_act(p)
                    pending.append(p)
                while pending and c >= PACK * pending[0] + 5:
                    pack_mm2(pending.pop(0))

            # U^T [EA, NSP] psum -> sbuf -> DRAM; divide/transpose on host.
            # Main part copied now (so next batch's mm2 can reuse its banks);
            # tail part deferred with the last pack.
            u_sb = u_pool.tile([EA, NSP], F32, tag="u", name=f"u{b}")
            nc.vector.tensor_copy(u_sb[:, 0:NMAIN], arena[:EA, U0:U0 + NMAIN])
            p_last = pending.pop()
            assert not pending

            # bind the batch-b tiles eagerly: xa_sb/pe_sbs are rebound
            # every batch iteration, and tail runs during batch b+1
            def tail(p=p_last, u=u_sb, b=b, xa_cur=xa_sb,
                     pe=pe_sbs.pop(p_last)):
                for i in range(PACK):
                    c = p * PACK + i
                    nc.tensor.matmul(
                        arena[:EA, UT0:UT0 + NT],
                        xa_cur[:, c, :],
                        pe[:LC, i * NT:(i + 1) * NT],
                        start=(c == 0), stop=(c == NLC - 1),
                    )
                nc.vector.tensor_copy(u[:, NMAIN:NSP], arena[:EA, UT0:UT0 + NT])
                nc.sync.dma_start(out=m_d[b], in_=u[:])

            prev_tail[0] = tail
        prev_tail[0]()
    nc.compile()
    return nc


def _get_nc():
    if not _NC:
        _NC.append(_build())
    return _NC[0]


def kernel(x, label_feature):
    global LAST_RESULT
    x = np.ascontiguousarray(np.asarray(x, dtype=np.float32))
    lf = np.ascontiguousarray(np.asarray(label_feature, dtype=np.float32))
    assert x.shape == (B, L, E) and lf.shape == (N_TOTAL, E)

    xa_f = np.zeros((B, LP, EA), np.float32)
    xa_f[:, :L, :E] = x
    xa_f[:, :L, E] = 1.0
    # [B, LP, EA] -> [B, LC, NLC, EA] so the device DMA is contiguous
    xa = np.ascontiguousarray(
        xa_f.reshape(B, NLC, LC, EA).transpose(0, 2, 1, 3)
    ).astype(ml_dtypes.bfloat16)
    xt = np.zeros((B, E, LP), np.float16)
    xt[:, :, :L] = x.transpose(0, 2, 1).astype(np.float16)

    in_maps = []
    for r in range(NCORES):
        lo = r * NS
        hi = min(lo + NS, N_TOTAL)
        labT = np.zeros((E, NSP), np.float16)
        labT[:, : hi - lo] = lf[lo:hi].T.astype(np.float16)
        in_maps.append({"xt": xt, "xa": xa, "labT": labT})

    nc = _get_nc()
    res = run_bass_kernel_spmd(
        nc, in_maps, core_ids=list(range(NCORES)), trace=TRACE
    )
    LAST_RESULT = res

    out = np.empty((B, N_TOTAL, E), np.float32)
    for r in range(NCORES):
        lo = r * NS
        hi = min(lo + NS, N_TOTAL)
        u = res.results[r]["m"]                      # [B, EA, NSP] f32
        m = u[:, :E, : hi - lo] / u[:, E:EA, : hi - lo]
        out[:, lo:hi, :] = m.transpose(0, 2, 1)
    return out
